# revision 25
# baseline (speedup 1.0000x reference)
"""Tensor-parallel causal GQA attention layer on 8 TRN2 NeuronCores.

Sharding: head-parallel. Core c computes q heads 4c..4c+3 and kv head c
(GQA groups align with the sharding), plus output-projection columns
[512c, 512c+512). The per-head attention outputs are AllGathered (bf16,
transposed layout) and each core computes its 512-column slice of the
output projection; the host concatenates the 8 column slices.

Device pipeline per 512-row sequence block:
  - stream x^T tiles (bf16), square+ones-matmul -> sum(x^2) -> rsqrt -> inv_rms
  - rope cos/sin tables on device (iota, Cody-Waite range reduction, ACT Sin),
    inv_rms folded into the tables; rotate-half head permutation is folded
    into the host-side weight layout so RoPE is 4 elementwise ops
  - Q/K projections in transposed [head_elem, seq] layout, V in natural layout
  - causal attention: scores^T = K^T_tile.T @ Q^T, exp on ACT (no max-sub:
    scores are bounded), block-skip above the diagonal, bf16 probabilities,
    colsum via ones-matmul, PV accumulation, deferred 1/colsum normalize
  - per-block AllGather of o^T, then the wo column-slice projection
"""
import sys

sys.path.insert(0, "/opt/trn_rl_repo")

import numpy as np
import ml_dtypes

import concourse.bass as bass
import concourse.mybir as mybir
import concourse.tile as tile
from concourse import bacc
from concourse import bass_utils
from concourse.alu_op_type import AluOpType

bf16 = ml_dtypes.bfloat16
F32 = mybir.dt.float32
BF = mybir.dt.bfloat16

S, D, H, KVH, HD = 2048, 4096, 32, 8, 128
EPS = 1e-5
NC_N = 8
HQ = H // NC_N              # 4 q heads per core
EC = HQ * HD                # 512
NB = 4                      # seq blocks of 512
BS = 512                    # block size
DT = D // 128               # 32 d-tiles
SCALE = float(128.0 ** -0.5)
TWO_PI = 6.283185307179586
C1 = float(np.float32(np.floor(TWO_PI * 2**11) / 2**11))
C2 = float(np.float32(np.floor((TWO_PI - C1) * 2**24) / 2**24))
C3 = float(np.float32(TWO_PI - C1 - C2))
PI = float(np.pi)

_NC_CACHE = None


def free_bcast(ap, n):
    """[P, 1] AP -> [P, n] with free-axis stride 0."""
    return bass.AP(ap.tensor, ap.offset, [[ap.ap[0][0], ap.ap[0][1]], [0, n]])


def build():
    nc = bacc.Bacc("TRN2", debug=False, num_devices=NC_N)

    xT = nc.dram_tensor("xT", [D, S], BF, kind="ExternalInput")
    wq = nc.dram_tensor("wq", [D, EC], BF, kind="ExternalInput")
    wk = nc.dram_tensor("wk", [D, HD], BF, kind="ExternalInput")
    wv = nc.dram_tensor("wv", [D, HD], BF, kind="ExternalInput")
    wo = nc.dram_tensor("wo", [D, EC], BF, kind="ExternalInput")
    invf2 = nc.dram_tensor("invf2", [128, 1], F32, kind="ExternalInput")
    maskB = nc.dram_tensor("maskB", [128, 896], BF, kind="ExternalInput")
    rsw = nc.dram_tensor("rsw", [128, 128], BF, kind="ExternalInput")
    out = nc.dram_tensor("out", [S, EC], F32, kind="ExternalOutput")

    scratch = nc.dram_tensor("scratch", [S], F32, kind="Internal")
    bounce = [nc.dram_tensor(f"bounce{b}", [EC, BS], BF, kind="Internal")
              for b in range(NB)]
    gath = [nc.dram_tensor(f"gath{b}", [D, BS], BF, kind="Internal",
                           addr_space="Shared") for b in range(NB)]

    with tile.TileContext(nc) as tc:
        from contextlib import ExitStack
        with ExitStack() as _es:
            wqp = _es.enter_context(tc.tile_pool(name="wqp", bufs=DT))
            wkp = _es.enter_context(tc.tile_pool(name="wkp", bufs=DT))
            wvp = _es.enter_context(tc.tile_pool(name="wvp", bufs=DT))
            wop = _es.enter_context(tc.tile_pool(name="wop", bufs=DT))
            xsp = _es.enter_context(tc.tile_pool(name="xs", bufs=DT + 1))
            xsqp = _es.enter_context(tc.tile_pool(name="xsq", bufs=2))
            cst = _es.enter_context(tc.tile_pool(name="cst", bufs=1))
            qtp = _es.enter_context(tc.tile_pool(name="qt", bufs=1))
            tmp = _es.enter_context(tc.tile_pool(name="tmp", bufs=6))
            tab = _es.enter_context(tc.tile_pool(name="tab", bufs=2))
            ropep = _es.enter_context(tc.tile_pool(name="rope", bufs=2))
            ptp = _es.enter_context(tc.tile_pool(name="pt", bufs=3))
            obp = _es.enter_context(tc.tile_pool(name="ob", bufs=2))
            ogp = _es.enter_context(tc.tile_pool(name="og", bufs=12))
            stgp = _es.enter_context(tc.tile_pool(name="stg", bufs=2))
            ps_acc = _es.enter_context(tc.tile_pool(name="ps_acc", bufs=3, space="PSUM"))
            ps_fast = _es.enter_context(tc.tile_pool(name="ps_fast", bufs=3, space="PSUM"))
            ps_small = _es.enter_context(tc.tile_pool(name="ps_small", bufs=2, space="PSUM"))
            # ---------------- activations first, then weights ----------------
            # xs tiles for block 0 go into the DMA queues ahead of the weight
            # slabs so the first ss/Q matmuls are not starved at startup.
            xs_pre = []
            for d in range(DT):
                t = xsp.tile([128, BS], BF, tag="xs", name=f"xs0_{d}")
                nc.sync.dma_start(t[:], xT.ap()[128 * d:128 * d + 128, 0:BS])
                xs_pre.append(t)
            wq_t = [wqp.tile([128, EC], BF, tag="wq", name=f"wq_{d}")
                    for d in range(DT)]
            wk_t = []
            wv_t = []
            wo_t = []
            for d in range(DT):
                nc.sync.dma_start(wq_t[d][:], wq.ap()[128 * d:128 * d + 128, :])
            for d in range(DT):
                t = wkp.tile([128, HD], BF, tag="wk")
                nc.sync.dma_start(t[:], wk.ap()[128 * d:128 * d + 128, :])
                wk_t.append(t)
                t = wvp.tile([128, HD], BF, tag="wv")
                nc.sync.dma_start(t[:], wv.ap()[128 * d:128 * d + 128, :])
                wv_t.append(t)
            ones_bf = cst.tile([128, 1], BF, tag="ones_bf")
            nc.vector.memset(ones_bf[:], 1.0)
            ones_row = cst.tile([1, 128], BF, tag="ones_row")
            nc.vector.memset(ones_row[:], 1.0)
            ones_row_f = cst.tile([1, 128], F32, tag="ones_row_f")
            nc.vector.memset(ones_row_f[:], 1.0)
            eps_t = cst.tile([128, 1], F32, tag="eps")
            nc.vector.memset(eps_t[:], EPS)
            invf_t = cst.tile([128, 1], F32, tag="invf")
            nc.sync.dma_start(invf_t[:], invf2.ap())
            maskB_t = cst.tile([128, 896], BF, tag="maskB")
            nc.sync.dma_start(maskB_t[:], maskB.ap())
            rsw_t = cst.tile([128, 128], BF, tag="rsw")
            nc.sync.dma_start(rsw_t[:], rsw.ap())

            # iota ramp 0..S-1 on every partition (Pool runs nothing after
            # the collectives; per-block tables slice this)
            R_all = cst.tile([128, S], F32, tag="R_all")
            nc.gpsimd.iota(R_all[:], pattern=[[1, S]], base=0,
                           channel_multiplier=0,
                           allow_small_or_imprecise_dtypes=True)

            # persistent activations
            Qt = [qtp.tile([128, S], BF, tag=f"Qt{h}", name=f"Qt{h}")
                  for h in range(HQ)]
            Kt = qtp.tile([128, S], BF, tag="Kt")
            Va = qtp.tile([128, S], BF, tag="Va")   # V in [s, e]: s-tile t at cols 128t

            pending_tails = [None, None]
            for b in range(NB):
                sblk = slice(BS * b, BS * b + BS)
                # ---- stream x^T slices for this block; squares + sum(x^2)
                xs = []
                ss_ps = ps_small.tile([1, BS], F32, tag="small", name=f"ss{b}")
                for d in range(DT):
                    if b == 0:
                        t = xs_pre[d]
                    else:
                        t = xsp.tile([128, BS], BF, tag="xs", name=f"xs{b}_{d}")
                        nc.sync.dma_start(t[:], xT.ap()[128 * d:128 * d + 128, sblk])
                    xs.append(t)
                    sq = xsqp.tile([128, BS], BF, tag="xsq")
                    nc.vector.tensor_tensor(sq[:], t[:], t[:], op=AluOpType.mult)
                    nc.tensor.matmul(ss_ps[:], ones_bf[:], sq[:],
                                     start=(d == 0), stop=(d == DT - 1))
                # inv_rms row for this block
                rms_sq = tmp.tile([1, BS], F32, tag="tscr")
                nc.scalar.activation(rms_sq[:], ss_ps[:],
                                     mybir.ActivationFunctionType.Sqrt,
                                     bias=eps_t[0:1, :], scale=1.0 / D)
                rms_row = tmp.tile([1, BS], F32, tag="tscr")
                nc.vector.reciprocal(rms_row[:], rms_sq[:])
                sc_slice = scratch.ap()[BS * b:BS * b + BS]
                nc.sync.dma_start(sc_slice, rms_row[:])
                rms_st = cst.tile([128, NB], F32, tag="rmsst", bufs=4, name=f"rmsst{b}")
                nc.sync.dma_start(
                    rms_st[:], bass.AP(scratch, sc_slice.offset, [[1, 128], [128, NB]]))
                rmsb_ps = ps_small.tile([128, BS], F32, tag="small",
                                        name=f"rmsb{b}")
                nc.tensor.matmul(rmsb_ps[:], ones_row_f[:], rms_row[:],
                                 start=True, stop=True)
                rms_b = tmp.tile([128, BS], F32, tag="tscr", name=f"rmsb_sb{b}")
                nc.scalar.copy(rms_b[:], rmsb_ps[:])

                # ---- rope tables for this block (inv_rms folded in)
                ang = tmp.tile([128, BS], F32, tag="tscr")
                nc.vector.tensor_tensor(ang[:], R_all[:, sblk],
                                        free_bcast(invf_t[:], BS),
                                        op=AluOpType.mult)
                kf0 = tmp.tile([128, BS], F32, tag="tscr")
                nc.vector.tensor_scalar_mul(kf0[:], ang[:], 1.0 / TWO_PI)
                ki = tmp.tile([128, BS], mybir.dt.int32, tag="tscr")
                nc.vector.tensor_copy(ki[:], kf0[:])
                kf = tmp.tile([128, BS], F32, tag="tscr")
                nc.vector.tensor_copy(kf[:], ki[:])
                red = tmp.tile([128, BS], F32, tag="tscr")
                nc.vector.cody_waite_cascade(red[:], ang[:], kf[:], C1, C2, C3)
                sarg = tmp.tile([128, BS], F32, tag="tscr")
                nc.vector.add_range_wrap(sarg[:], red[:], 0.0, PI, TWO_PI)
                carg = tmp.tile([128, BS], F32, tag="tscr")
                nc.vector.add_range_wrap(carg[:], red[:], PI / 2, PI, TWO_PI)
                sn = tmp.tile([128, BS], F32, tag="tscr")
                nc.scalar.activation(sn[:], sarg[:], mybir.ActivationFunctionType.Sin)
                cs = tmp.tile([128, BS], F32, tag="tscr")
                nc.scalar.activation(cs[:], carg[:], mybir.ActivationFunctionType.Sin)
                csp = tab.tile([128, BS], F32, tag="csp")
                nc.vector.tensor_tensor(csp[:], cs[:], rms_b[:], op=AluOpType.mult)
                snp = tab.tile([128, BS], F32, tag="snp")
                nc.vector.tensor_tensor(snp[:], sn[:], rms_b[:], op=AluOpType.mult)

                # ---- Q/K projections (transposed layout) + fused rope
                def rope_out(ps, dst, _n=[0]):
                    # q_hat = q*cos + signed_halfswap(q*sin); the swap+sign is
                    # one PE matmul against a constant permutation (rsw)
                    A = ropep.tile([128, BS], F32, tag="ropeA")
                    nc.vector.tensor_tensor(A[:], ps[:], csp[:], op=AluOpType.mult)
                    Bt = ropep.tile([128, BS], BF, tag="ropeB")
                    nc.vector.tensor_tensor(Bt[:], ps[:], snp[:], op=AluOpType.mult)
                    _n[0] += 1
                    bsw = ps_small.tile([128, BS], F32, tag="small",
                                        name=f"bsw{b}_{_n[0]}")
                    nc.tensor.matmul(bsw[:], rsw_t[:], Bt[:], start=True, stop=True)
                    nc.vector.tensor_tensor(dst[:, sblk], A[:], bsw[:],
                                            op=AluOpType.add)

                for h in range(HQ):
                    q_ps = ps_acc.tile([128, BS], F32, tag="acc")
                    for d in range(DT):
                        nc.tensor.matmul(q_ps[:], wq_t[d][:, 128 * h:128 * h + 128],
                                         xs[d][:], start=(d == 0), stop=(d == DT - 1))
                    rope_out(q_ps, Qt[h])
                k_ps = ps_acc.tile([128, BS], F32, tag="acc", name=f"kps{b}")
                for d in range(DT):
                    nc.tensor.matmul(k_ps[:], wk_t[d][:], xs[d][:],
                                     start=(d == 0), stop=(d == DT - 1))
                rope_out(k_ps, Kt)

                # ---- V projection (natural layout) + inv_rms scale
                for st in range(4):
                    v_ps = ps_acc.tile([128, HD], F32, tag="acc")
                    for d in range(DT):
                        nc.tensor.matmul(v_ps[:], xs[d][:, 128 * st:128 * st + 128],
                                         wv_t[d][:], start=(d == 0), stop=(d == DT - 1))
                    col = 128 * (4 * b + st)
                    nc.vector.tensor_tensor(
                        Va[:, col:col + 128], v_ps[:],
                        free_bcast(rms_st[:, st:st + 1], 128), op=AluOpType.mult)

                # ---- causal attention (deferred normalize tails)
                jmax = 4 * b + 3

                def flush_tails():
                    for i in range(2):
                        if pending_tails[i] is not None:
                            pending_tails[i]()
                            pending_tails[i] = None

                for h in range(HQ):
                    sc = {}

                    def issue_score(j, h=h):
                        sc[j] = ps_fast.tile([128, BS], F32, tag="sc",
                                             name=f"sc{b}_{h}_{j}")
                        nc.tensor.matmul(sc[j][:],
                                         Kt[:, 128 * j:128 * j + 128],
                                         Qt[h][:, sblk], start=True, stop=True)

                    issue_score(0)
                    flush_tails()
                    if jmax >= 1:
                        issue_score(1)
                    o_ps = ps_acc.tile([128, BS], F32, tag="acc",
                                       name=f"ops{b}_{h}")
                    cs_ps = ps_small.tile([1, BS], F32, tag="small",
                                          name=f"cs{b}_{h}")
                    for j in range(jmax + 1):
                        P = ptp.tile([128, BS], BF, tag="P", name=f"P{b}_{h}_{j}")
                        nc.scalar.activation(P[:], sc.pop(j)[:],
                                             mybir.ActivationFunctionType.Exp,
                                             scale=SCALE)
                        if j + 2 <= jmax:
                            issue_score(j + 2)
                        jj = j - 4 * b
                        if jj >= 0:
                            off = 384 - 128 * jj
                            nc.vector.tensor_tensor(P[:], P[:],
                                                    maskB_t[:, off:off + BS],
                                                    op=AluOpType.mult)
                        nc.tensor.matmul(cs_ps[:], ones_bf[:], P[:],
                                         start=(j == 0), stop=(j == jmax))
                        nc.tensor.matmul(o_ps[:], Va[:, 128 * j:128 * j + 128],
                                         P[:], start=(j == 0), stop=(j == jmax))

                    def make_tail(h, o_ps_h, cs_ps_h, b=b):
                        def tail():
                            inv_f = tmp.tile([1, BS], F32, tag="invf_r", bufs=2,
                                             name=f"invf{b}_{h}")
                            nc.vector.reciprocal(inv_f[:], cs_ps_h[:])
                            inv_b = tmp.tile([1, BS], BF, tag="invb_r", bufs=2,
                                             name=f"invb{b}_{h}")
                            nc.vector.tensor_copy(inv_b[:], inv_f[:])
                            bc_ps = ps_small.tile([128, BS], F32, tag="small",
                                                  name=f"bc{b}_{h}")
                            nc.tensor.matmul(bc_ps[:], ones_row[:], inv_b[:],
                                             start=True, stop=True)
                            invb_sb = stgp.tile([128, BS], BF, tag="invsb",
                                                name=f"invsb{b}_{h}")
                            nc.scalar.copy(invb_sb[:], bc_ps[:])
                            obuf = obp.tile([128, BS], BF, tag="obuf",
                                            name=f"obuf{b}_{h}")
                            nc.vector.tensor_tensor(obuf[:], o_ps_h[:],
                                                    invb_sb[:], op=AluOpType.mult)
                            nc.sync.dma_start(
                                bounce[b].ap()[128 * h:128 * h + 128, :], obuf[:])
                        return tail

                    pending_tails[h % 2] = make_tail(h, o_ps, cs_ps)
                flush_tails()
                nc.gpsimd.collective_compute(
                    "AllGather", AluOpType.bypass,
                    replica_groups=[list(range(NC_N))],
                    ins=[bounce[b].ap().opt()],
                    outs=[gath[b].ap().opt()],
                )

            # ---------------- output projection per block ----------------
            for d in range(DT):
                t = wop.tile([128, EC], BF, tag="wo", name=f"wo_{d}")
                nc.sync.dma_start(t[:], wo.ap()[128 * d:128 * d + 128, :])
                wo_t.append(t)
            for b in range(NB):
                out_ps = [(ps_acc if st < 3 else ps_fast).tile(
                    [128, EC], F32, tag=("acc" if st < 3 else "sc"),
                    name=f"out_ps{b}_{st}") for st in range(4)]
                for j in range(DT):
                    og = ogp.tile([128, BS], BF, tag="og", name=f"og{b}_{j}")
                    nc.sync.dma_start(og[:], gath[b].ap()[128 * j:128 * j + 128, :])
                    for st in range(4):
                        nc.tensor.matmul(out_ps[st][:], og[:, 128 * st:128 * st + 128],
                                         wo_t[j][:], start=(j == 0), stop=(j == DT - 1))
                for st in range(4):
                    stage = stgp.tile([128, EC], F32, tag="stage")
                    nc.scalar.copy(stage[:], out_ps[st][:])
                    row = BS * b + 128 * st
                    nc.sync.dma_start(out.ap()[row:row + 128, :], stage[:])

    nc.finalize()
    return nc


def _host_prep(current_embedding, norm_weight, wq, wk, wv, wo, rope_freqs):
    x = np.asarray(current_embedding, np.float32)
    nw = np.asarray(norm_weight, np.float32)
    wq = np.asarray(wq, np.float32)
    wk = np.asarray(wk, np.float32)
    wv = np.asarray(wv, np.float32)
    wo = np.asarray(wo, np.float32)
    rf = np.asarray(rope_freqs, np.float32)

    xT = np.ascontiguousarray(x.T).astype(bf16)
    perm = np.concatenate([np.arange(0, HD, 2), np.arange(1, HD, 2)])
    wq_p = (wq * nw[None, :]).reshape(H, HD, D)[:, perm, :]
    wk_p = (wk * nw[None, :]).reshape(KVH, HD, D)[:, perm, :]
    wv_f = (wv * nw[None, :]).reshape(KVH, HD, D)
    woT = np.ascontiguousarray(wo.T).astype(bf16)
    invf2 = np.concatenate([rf, rf]).astype(np.float32)[:, None]
    B = (np.arange(896)[None, :] - 384 >= np.arange(128)[:, None]).astype(bf16)
    RSW = np.zeros((128, 128), dtype=bf16)
    RSW[np.arange(64) + 64, np.arange(64)] = -1.0
    RSW[np.arange(64), np.arange(64) + 64] = 1.0

    in_maps = []
    for c in range(NC_N):
        in_maps.append({
            "xT": xT,
            "wq": np.ascontiguousarray(
                wq_p[HQ * c:HQ * c + HQ].reshape(EC, D).T).astype(bf16),
            "wk": np.ascontiguousarray(wk_p[c].T).astype(bf16),
            "wv": np.ascontiguousarray(wv_f[c].T).astype(bf16),
            "wo": np.ascontiguousarray(woT[:, EC * c:EC * c + EC]),
            "invf2": invf2,
            "maskB": B,
            "rsw": RSW,
        })
    return in_maps


def get_nc():
    global _NC_CACHE
    if _NC_CACHE is None:
        _NC_CACHE = build()
    return _NC_CACHE


def run(in_maps, **kwargs):
    return bass_utils.run_bass_kernel_spmd(
        get_nc(), in_maps, core_ids=list(range(NC_N)), **kwargs)


def kernel(current_embedding, norm_weight, wq, wk, wv, wo, rope_freqs):
    in_maps = _host_prep(current_embedding, norm_weight, wq, wk, wv, wo,
                         rope_freqs)
    res = run(in_maps)
    return np.hstack([res.results[c]["out"] for c in range(NC_N)])


# revision 30
# speedup vs baseline: 1.0017x; 1.0017x over previous
"""Tensor-parallel causal GQA attention layer on 8 TRN2 NeuronCores.

Sharding: head-parallel. Core c computes q heads 4c..4c+3 and kv head c
(GQA groups align with the sharding), plus output-projection columns
[512c, 512c+512). The per-head attention outputs are AllGathered (bf16,
transposed layout) and each core computes its 512-column slice of the
output projection; the host concatenates the 8 column slices.

Device pipeline per 512-row sequence block:
  - stream x^T tiles (bf16), square+ones-matmul -> sum(x^2) -> rsqrt -> inv_rms
  - rope cos/sin tables on device (iota, Cody-Waite range reduction, ACT Sin),
    inv_rms folded into the tables; rotate-half head permutation is folded
    into the host-side weight layout so RoPE is 4 elementwise ops
  - Q/K projections in transposed [head_elem, seq] layout, V in natural layout
  - causal attention: scores^T = K^T_tile.T @ Q^T, exp on ACT (no max-sub:
    scores are bounded), block-skip above the diagonal, bf16 probabilities,
    colsum via ones-matmul, PV accumulation, deferred 1/colsum normalize
  - per-block AllGather of o^T, then the wo column-slice projection
"""
import sys

sys.path.insert(0, "/opt/trn_rl_repo")

import numpy as np
import ml_dtypes

import concourse.bass as bass
import concourse.mybir as mybir
import concourse.tile as tile
from concourse import bacc
from concourse import bass_utils
from concourse.alu_op_type import AluOpType

bf16 = ml_dtypes.bfloat16
F32 = mybir.dt.float32
BF = mybir.dt.bfloat16

S, D, H, KVH, HD = 2048, 4096, 32, 8, 128
EPS = 1e-5
NC_N = 8
HQ = H // NC_N              # 4 q heads per core
EC = HQ * HD                # 512
NB = 4                      # seq blocks of 512
BS = 512                    # block size
DT = D // 128               # 32 d-tiles
SCALE = float(128.0 ** -0.5)
TWO_PI = 6.283185307179586
C1 = float(np.float32(np.floor(TWO_PI * 2**11) / 2**11))
C2 = float(np.float32(np.floor((TWO_PI - C1) * 2**24) / 2**24))
C3 = float(np.float32(TWO_PI - C1 - C2))
PI = float(np.pi)

_NC_CACHE = None


def free_bcast(ap, n):
    """[P, 1] AP -> [P, n] with free-axis stride 0."""
    return bass.AP(ap.tensor, ap.offset, [[ap.ap[0][0], ap.ap[0][1]], [0, n]])


def build():
    nc = bacc.Bacc("TRN2", debug=False, num_devices=NC_N)

    xT = nc.dram_tensor("xT", [D, S], BF, kind="ExternalInput")
    wq = nc.dram_tensor("wq", [D, EC], BF, kind="ExternalInput")
    wk = nc.dram_tensor("wk", [D, HD], BF, kind="ExternalInput")
    wv = nc.dram_tensor("wv", [D, HD], BF, kind="ExternalInput")
    wo = nc.dram_tensor("wo", [D, EC], BF, kind="ExternalInput")
    invf2 = nc.dram_tensor("invf2", [128, 1], F32, kind="ExternalInput")
    maskB = nc.dram_tensor("maskB", [128, 896], BF, kind="ExternalInput")
    rsw = nc.dram_tensor("rsw", [128, 128], BF, kind="ExternalInput")
    out = nc.dram_tensor("out", [S, EC], F32, kind="ExternalOutput")

    scratch = nc.dram_tensor("scratch", [S], F32, kind="Internal")
    bounce = [nc.dram_tensor(f"bounce{b}", [EC, BS], BF, kind="Internal")
              for b in range(NB)]
    gath = [nc.dram_tensor(f"gath{b}", [D, BS], BF, kind="Internal",
                           addr_space="Shared") for b in range(NB)]

    with tile.TileContext(nc) as tc:
        from contextlib import ExitStack
        with ExitStack() as _es:
            wqp = _es.enter_context(tc.tile_pool(name="wqp", bufs=DT))
            wkp = _es.enter_context(tc.tile_pool(name="wkp", bufs=DT))
            wvp = _es.enter_context(tc.tile_pool(name="wvp", bufs=DT))
            wop = _es.enter_context(tc.tile_pool(name="wop", bufs=DT))
            xsp = _es.enter_context(tc.tile_pool(name="xs", bufs=DT + 1))
            xsqp = _es.enter_context(tc.tile_pool(name="xsq", bufs=2))
            cst = _es.enter_context(tc.tile_pool(name="cst", bufs=1))
            qtp = _es.enter_context(tc.tile_pool(name="qt", bufs=1))
            tmp = _es.enter_context(tc.tile_pool(name="tmp", bufs=6))
            tab = _es.enter_context(tc.tile_pool(name="tab", bufs=2))
            ropep = _es.enter_context(tc.tile_pool(name="rope", bufs=2))
            ptp = _es.enter_context(tc.tile_pool(name="pt", bufs=4))
            obp = _es.enter_context(tc.tile_pool(name="ob", bufs=3))
            ogp = _es.enter_context(tc.tile_pool(name="og", bufs=12))
            stgp = _es.enter_context(tc.tile_pool(name="stg", bufs=2))
            ps_acc = _es.enter_context(tc.tile_pool(name="ps_acc", bufs=3, space="PSUM"))
            ps_fast = _es.enter_context(tc.tile_pool(name="ps_fast", bufs=3, space="PSUM"))
            ps_small = _es.enter_context(tc.tile_pool(name="ps_small", bufs=2, space="PSUM"))
            # ---------------- activations first, then weights ----------------
            # xs tiles for block 0 go into the DMA queues ahead of the weight
            # slabs so the first ss/Q matmuls are not starved at startup.
            xs_pre = []
            for d in range(DT):
                t = xsp.tile([128, BS], BF, tag="xs", name=f"xs0_{d}")
                nc.sync.dma_start(t[:], xT.ap()[128 * d:128 * d + 128, 0:BS])
                xs_pre.append(t)
            wq_t = [wqp.tile([128, EC], BF, tag="wq", name=f"wq_{d}")
                    for d in range(DT)]
            wk_t = []
            wv_t = []
            wo_t = []
            for d in range(DT):
                nc.sync.dma_start(wq_t[d][:], wq.ap()[128 * d:128 * d + 128, :])
            for d in range(DT):
                t = wkp.tile([128, HD], BF, tag="wk")
                nc.sync.dma_start(t[:], wk.ap()[128 * d:128 * d + 128, :])
                wk_t.append(t)
                t = wvp.tile([128, HD], BF, tag="wv")
                nc.sync.dma_start(t[:], wv.ap()[128 * d:128 * d + 128, :])
                wv_t.append(t)
            ones_bf = cst.tile([128, 1], BF, tag="ones_bf")
            nc.vector.memset(ones_bf[:], 1.0)
            ones_row = cst.tile([1, 128], BF, tag="ones_row")
            nc.vector.memset(ones_row[:], 1.0)
            ones_row_f = cst.tile([1, 128], F32, tag="ones_row_f")
            nc.vector.memset(ones_row_f[:], 1.0)
            eps_t = cst.tile([128, 1], F32, tag="eps")
            nc.vector.memset(eps_t[:], EPS)
            invf_t = cst.tile([128, 1], F32, tag="invf")
            nc.sync.dma_start(invf_t[:], invf2.ap())
            maskB_t = cst.tile([128, 896], BF, tag="maskB")
            nc.sync.dma_start(maskB_t[:], maskB.ap())
            rsw_t = cst.tile([128, 128], BF, tag="rsw")
            nc.sync.dma_start(rsw_t[:], rsw.ap())

            # iota ramp 0..S-1 on every partition (Pool runs nothing after
            # the collectives; per-block tables slice this)
            R_all = cst.tile([128, S], F32, tag="R_all")
            nc.gpsimd.iota(R_all[:], pattern=[[1, S]], base=0,
                           channel_multiplier=0,
                           allow_small_or_imprecise_dtypes=True)

            # persistent activations
            Qt = [qtp.tile([128, S], BF, tag=f"Qt{h}", name=f"Qt{h}")
                  for h in range(HQ)]
            Kt = qtp.tile([128, S], BF, tag="Kt")
            Va = qtp.tile([128, S], BF, tag="Va")   # V in [s, e]: s-tile t at cols 128t

            pending_tails = [None, None]
            for b in range(NB):
                sblk = slice(BS * b, BS * b + BS)
                # ---- stream x^T slices for this block; squares + sum(x^2)
                xs = []
                ss_ps = ps_small.tile([1, BS], F32, tag="small", name=f"ss{b}")
                for d in range(DT):
                    if b == 0:
                        t = xs_pre[d]
                    else:
                        t = xsp.tile([128, BS], BF, tag="xs", name=f"xs{b}_{d}")
                        nc.sync.dma_start(t[:], xT.ap()[128 * d:128 * d + 128, sblk])
                    xs.append(t)
                    sq = xsqp.tile([128, BS], BF, tag="xsq")
                    nc.vector.tensor_tensor(sq[:], t[:], t[:], op=AluOpType.mult)
                    nc.tensor.matmul(ss_ps[:], ones_bf[:], sq[:],
                                     start=(d == 0), stop=(d == DT - 1))
                # inv_rms row for this block
                rms_sq = tmp.tile([1, BS], F32, tag="tscr")
                nc.scalar.activation(rms_sq[:], ss_ps[:],
                                     mybir.ActivationFunctionType.Sqrt,
                                     bias=eps_t[0:1, :], scale=1.0 / D)
                rms_row = tmp.tile([1, BS], F32, tag="tscr")
                nc.vector.reciprocal(rms_row[:], rms_sq[:])
                sc_slice = scratch.ap()[BS * b:BS * b + BS]
                nc.sync.dma_start(sc_slice, rms_row[:])
                rms_st = cst.tile([128, NB], F32, tag="rmsst", bufs=4, name=f"rmsst{b}")
                nc.sync.dma_start(
                    rms_st[:], bass.AP(scratch, sc_slice.offset, [[1, 128], [128, NB]]))
                rmsb_ps = ps_small.tile([128, BS], F32, tag="small",
                                        name=f"rmsb{b}")
                nc.tensor.matmul(rmsb_ps[:], ones_row_f[:], rms_row[:],
                                 start=True, stop=True)
                rms_b = tmp.tile([128, BS], F32, tag="tscr", name=f"rmsb_sb{b}")
                nc.scalar.copy(rms_b[:], rmsb_ps[:])

                # ---- rope tables for this block (inv_rms folded in)
                ang = tmp.tile([128, BS], F32, tag="tscr")
                nc.vector.tensor_tensor(ang[:], R_all[:, sblk],
                                        free_bcast(invf_t[:], BS),
                                        op=AluOpType.mult)
                kf0 = tmp.tile([128, BS], F32, tag="tscr")
                nc.vector.tensor_scalar_mul(kf0[:], ang[:], 1.0 / TWO_PI)
                ki = tmp.tile([128, BS], mybir.dt.int32, tag="tscr")
                nc.vector.tensor_copy(ki[:], kf0[:])
                kf = tmp.tile([128, BS], F32, tag="tscr")
                nc.vector.tensor_copy(kf[:], ki[:])
                red = tmp.tile([128, BS], F32, tag="tscr")
                nc.vector.cody_waite_cascade(red[:], ang[:], kf[:], C1, C2, C3)
                sarg = tmp.tile([128, BS], F32, tag="tscr")
                nc.vector.add_range_wrap(sarg[:], red[:], 0.0, PI, TWO_PI)
                carg = tmp.tile([128, BS], F32, tag="tscr")
                nc.vector.add_range_wrap(carg[:], red[:], PI / 2, PI, TWO_PI)
                sn = tmp.tile([128, BS], F32, tag="tscr")
                nc.scalar.activation(sn[:], sarg[:], mybir.ActivationFunctionType.Sin)
                cs = tmp.tile([128, BS], F32, tag="tscr")
                nc.scalar.activation(cs[:], carg[:], mybir.ActivationFunctionType.Sin)
                csp = tab.tile([128, BS], F32, tag="csp")
                nc.vector.tensor_tensor(csp[:], cs[:], rms_b[:], op=AluOpType.mult)
                snp = tab.tile([128, BS], F32, tag="snp")
                nc.vector.tensor_tensor(snp[:], sn[:], rms_b[:], op=AluOpType.mult)

                # ---- Q/K projections (transposed layout) + fused rope
                def rope_out(ps, dst, _n=[0]):
                    # q_hat = q*cos + signed_halfswap(q*sin); the swap+sign is
                    # one PE matmul against a constant permutation (rsw)
                    A = ropep.tile([128, BS], F32, tag="ropeA")
                    nc.vector.tensor_tensor(A[:], ps[:], csp[:], op=AluOpType.mult)
                    Bt = ropep.tile([128, BS], BF, tag="ropeB")
                    nc.vector.tensor_tensor(Bt[:], ps[:], snp[:], op=AluOpType.mult)
                    _n[0] += 1
                    bsw = ps_small.tile([128, BS], F32, tag="small",
                                        name=f"bsw{b}_{_n[0]}")
                    nc.tensor.matmul(bsw[:], rsw_t[:], Bt[:], start=True, stop=True)
                    nc.vector.tensor_tensor(dst[:, sblk], A[:], bsw[:],
                                            op=AluOpType.add)

                for h in range(HQ):
                    q_ps = ps_acc.tile([128, BS], F32, tag="acc")
                    for d in range(DT):
                        nc.tensor.matmul(q_ps[:], wq_t[d][:, 128 * h:128 * h + 128],
                                         xs[d][:], start=(d == 0), stop=(d == DT - 1))
                    rope_out(q_ps, Qt[h])
                k_ps = ps_acc.tile([128, BS], F32, tag="acc", name=f"kps{b}")
                for d in range(DT):
                    nc.tensor.matmul(k_ps[:], wk_t[d][:], xs[d][:],
                                     start=(d == 0), stop=(d == DT - 1))
                rope_out(k_ps, Kt)

                # ---- V projection (natural layout) + inv_rms scale
                for st in range(4):
                    v_ps = ps_acc.tile([128, HD], F32, tag="acc")
                    for d in range(DT):
                        nc.tensor.matmul(v_ps[:], xs[d][:, 128 * st:128 * st + 128],
                                         wv_t[d][:], start=(d == 0), stop=(d == DT - 1))
                    col = 128 * (4 * b + st)
                    nc.vector.tensor_tensor(
                        Va[:, col:col + 128], v_ps[:],
                        free_bcast(rms_st[:, st:st + 1], 128), op=AluOpType.mult)

                # ---- causal attention (deferred normalize tails)
                jmax = 4 * b + 3

                def flush_tails():
                    for i in range(2):
                        if pending_tails[i] is not None:
                            pending_tails[i]()
                            pending_tails[i] = None

                for h in range(HQ):
                    sc = {}

                    def issue_score(j, h=h):
                        sc[j] = ps_fast.tile([128, BS], F32, tag="sc",
                                             name=f"sc{b}_{h}_{j}")
                        nc.tensor.matmul(sc[j][:],
                                         Kt[:, 128 * j:128 * j + 128],
                                         Qt[h][:, sblk], start=True, stop=True)

                    issue_score(0)
                    flush_tails()
                    if jmax >= 1:
                        issue_score(1)
                    o_ps = ps_acc.tile([128, BS], F32, tag="acc",
                                       name=f"ops{b}_{h}")
                    cs_ps = ps_small.tile([1, BS], F32, tag="small",
                                          name=f"cs{b}_{h}")
                    for j in range(jmax + 1):
                        P = ptp.tile([128, BS], BF, tag="P", name=f"P{b}_{h}_{j}")
                        nc.scalar.activation(P[:], sc.pop(j)[:],
                                             mybir.ActivationFunctionType.Exp,
                                             scale=SCALE)
                        if j + 2 <= jmax:
                            issue_score(j + 2)
                        jj = j - 4 * b
                        if jj >= 0:
                            off = 384 - 128 * jj
                            nc.vector.tensor_tensor(P[:], P[:],
                                                    maskB_t[:, off:off + BS],
                                                    op=AluOpType.mult)
                        nc.tensor.matmul(cs_ps[:], ones_bf[:], P[:],
                                         start=(j == 0), stop=(j == jmax))
                        nc.tensor.matmul(o_ps[:], Va[:, 128 * j:128 * j + 128],
                                         P[:], start=(j == 0), stop=(j == jmax))

                    def make_tail(h, o_ps_h, cs_ps_h, b=b):
                        def tail():
                            inv_f = tmp.tile([1, BS], F32, tag="invf_r", bufs=2,
                                             name=f"invf{b}_{h}")
                            nc.vector.reciprocal(inv_f[:], cs_ps_h[:])
                            inv_b = tmp.tile([1, BS], BF, tag="invb_r", bufs=2,
                                             name=f"invb{b}_{h}")
                            nc.vector.tensor_copy(inv_b[:], inv_f[:])
                            bc_ps = ps_small.tile([128, BS], F32, tag="small",
                                                  name=f"bc{b}_{h}")
                            nc.tensor.matmul(bc_ps[:], ones_row[:], inv_b[:],
                                             start=True, stop=True)
                            invb_sb = stgp.tile([128, BS], BF, tag="invsb",
                                                name=f"invsb{b}_{h}")
                            nc.scalar.copy(invb_sb[:], bc_ps[:])
                            obuf = obp.tile([128, BS], BF, tag="obuf",
                                            name=f"obuf{b}_{h}")
                            nc.vector.tensor_tensor(obuf[:], o_ps_h[:],
                                                    invb_sb[:], op=AluOpType.mult)
                            nc.sync.dma_start(
                                bounce[b].ap()[128 * h:128 * h + 128, :], obuf[:])
                        return tail

                    pending_tails[h % 2] = make_tail(h, o_ps, cs_ps)
                flush_tails()
                nc.gpsimd.collective_compute(
                    "AllGather", AluOpType.bypass,
                    replica_groups=[list(range(NC_N))],
                    ins=[bounce[b].ap().opt()],
                    outs=[gath[b].ap().opt()],
                )

            # ---------------- output projection per block ----------------
            for d in range(DT):
                t = wop.tile([128, EC], BF, tag="wo", name=f"wo_{d}")
                nc.sync.dma_start(t[:], wo.ap()[128 * d:128 * d + 128, :])
                wo_t.append(t)
            for b in range(NB):
                out_ps = [(ps_acc if st < 3 else ps_fast).tile(
                    [128, EC], F32, tag=("acc" if st < 3 else "sc"),
                    name=f"out_ps{b}_{st}") for st in range(4)]
                for j in range(DT):
                    og = ogp.tile([128, BS], BF, tag="og", name=f"og{b}_{j}")
                    nc.sync.dma_start(og[:], gath[b].ap()[128 * j:128 * j + 128, :])
                    for st in range(4):
                        nc.tensor.matmul(out_ps[st][:], og[:, 128 * st:128 * st + 128],
                                         wo_t[j][:], start=(j == 0), stop=(j == DT - 1))
                for st in range(4):
                    stage = stgp.tile([128, EC], F32, tag="stage")
                    nc.scalar.copy(stage[:], out_ps[st][:])
                    row = BS * b + 128 * st
                    nc.sync.dma_start(out.ap()[row:row + 128, :], stage[:])

    nc.finalize()
    return nc


def _host_prep(current_embedding, norm_weight, wq, wk, wv, wo, rope_freqs):
    x = np.asarray(current_embedding, np.float32)
    nw = np.asarray(norm_weight, np.float32)
    wq = np.asarray(wq, np.float32)
    wk = np.asarray(wk, np.float32)
    wv = np.asarray(wv, np.float32)
    wo = np.asarray(wo, np.float32)
    rf = np.asarray(rope_freqs, np.float32)

    xT = np.ascontiguousarray(x.T).astype(bf16)
    perm = np.concatenate([np.arange(0, HD, 2), np.arange(1, HD, 2)])
    wq_p = (wq * nw[None, :]).reshape(H, HD, D)[:, perm, :]
    wk_p = (wk * nw[None, :]).reshape(KVH, HD, D)[:, perm, :]
    wv_f = (wv * nw[None, :]).reshape(KVH, HD, D)
    woT = np.ascontiguousarray(wo.T).astype(bf16)
    invf2 = np.concatenate([rf, rf]).astype(np.float32)[:, None]
    B = (np.arange(896)[None, :] - 384 >= np.arange(128)[:, None]).astype(bf16)
    RSW = np.zeros((128, 128), dtype=bf16)
    RSW[np.arange(64) + 64, np.arange(64)] = -1.0
    RSW[np.arange(64), np.arange(64) + 64] = 1.0

    in_maps = []
    for c in range(NC_N):
        in_maps.append({
            "xT": xT,
            "wq": np.ascontiguousarray(
                wq_p[HQ * c:HQ * c + HQ].reshape(EC, D).T).astype(bf16),
            "wk": np.ascontiguousarray(wk_p[c].T).astype(bf16),
            "wv": np.ascontiguousarray(wv_f[c].T).astype(bf16),
            "wo": np.ascontiguousarray(woT[:, EC * c:EC * c + EC]),
            "invf2": invf2,
            "maskB": B,
            "rsw": RSW,
        })
    return in_maps


def get_nc():
    global _NC_CACHE
    if _NC_CACHE is None:
        _NC_CACHE = build()
    return _NC_CACHE


def run(in_maps, **kwargs):
    return bass_utils.run_bass_kernel_spmd(
        get_nc(), in_maps, core_ids=list(range(NC_N)), **kwargs)


def kernel(current_embedding, norm_weight, wq, wk, wv, wo, rope_freqs):
    in_maps = _host_prep(current_embedding, norm_weight, wq, wk, wv, wo,
                         rope_freqs)
    res = run(in_maps)
    return np.hstack([res.results[c]["out"] for c in range(NC_N)])
